# revision 13
# baseline (speedup 1.0000x reference)
"""
Trainium2 Bass kernel for nn_Encoder (embedding lookup + LSTM, returns final (h, c)).

Strategy (data-parallel over batch, per sharding hint):
  - 8 cores, each handles B_local = 4 of the 32 batch rows.
  - Per core: gather embedding rows via indirect DMA (t-major order),
    transpose on PE, project x @ W with fp32r matmuls (chunked over T),
    then run the 512-step recurrence with U as the stationary operand
    in fp16 (FWL 2x weight loads) producing gates transposed
    (4H on partitions) so activations/cell update run wide on ACT/DVE.
  - Gate layout: psum z tile per H-slice hs (4 of them, one PSUM bank each),
    packed columns (gate', b) with gate' order (i, f, o, g) so one sigmoid
    covers i,f,o and one tanh covers g.
  - h is kept as hT [128 x (hs, b)] fp16 which is exactly the moving-operand
    layout the next step's matmuls need.

Host side: shard/marshal inputs, run SPMD on 8 cores, unpack outputs.
"""

import numpy as np

import concourse.bass as bass
import concourse.mybir as mybir
import concourse.tile as tile
from concourse import bacc
from concourse.bass import IndirectOffsetOnAxis
from concourse.bass_utils import run_bass_kernel_spmd
from concourse.masks import make_identity

# Problem constants (hardcoded; harness contract)
B, T, V, E, H = 32, 512, 20000, 300, 512
G4 = 4 * H            # 2048
NCORES = 8
BL = B // NCORES      # 4 batch rows per core
P = 128
KM = G4 // P          # 16 M-tiles over 4H
KH = H // P           # 4 K-tiles over H
KE_SIZES = [128, 128, 44]   # K subtiles over E=300
# Keras gate g (i,f,g,o) -> packed slot (i,f,o,g): sigmoid = slots 0..2, tanh = slot 3
PERM = [0, 1, 3, 2]

f32 = mybir.dt.float32
f32r = mybir.dt.float32r
f16 = mybir.dt.float16
f8 = mybir.dt.float8e3
i32 = mybir.dt.int32

# U (and W, b) are pre-scaled by USC on the host so that U fits the fp8e3
# (e3m4) normal range: raw |U| ~ N(0, 1/sqrt(H)) lies below the 0.25 min
# normal, 32x shifts it into [~0.25, 15.5]. z comes out of PSUM scaled by
# USC; the gate activations undo it via their input scale (exact power of 2).
USC = 32.0

AF = mybir.ActivationFunctionType


def build_program(nc, T_steps=T, Tc=128, dbg_step=None, reps=1, sched="v2"):
    """Emit the full per-core program into nc (a bacc.Bacc).

    reps > 1 repeats the whole compute (for timing amplification)."""
    assert T_steps % Tc == 0
    nch = T_steps // Tc
    NJ = Tc * BL // P  # gathers (128-row groups) per chunk

    emb_t = nc.declare_dram_parameter("emb", [V, E], f32, isOutput=False)
    W_t = nc.declare_dram_parameter("W", [E, G4], f32, isOutput=False)
    U_t = nc.declare_dram_parameter("U", [H, G4], f32, isOutput=False)
    b_t = nc.declare_dram_parameter("bvec", [G4], f32, isOutput=False)
    tok_t = nc.declare_dram_parameter("tok", [P, T_steps * BL // P], i32, isOutput=False)
    ho_t = nc.declare_dram_parameter("ho", [P, BL * KH], f16, isOutput=True)
    co_t = nc.declare_dram_parameter("co", [P, BL * KH], f32, isOutput=True)
    if dbg_step is not None:
        dbg_z = nc.declare_dram_parameter("dbg_z", [P, 64], f32, isOutput=True)
        dbg_h = nc.declare_dram_parameter("dbg_h", [P, BL * KH], f16, isOutput=True)
        dbg_c = nc.declare_dram_parameter("dbg_c", [P, BL * KH], f32, isOutput=True)

    with tile.TileContext(nc) as tc:
        with (
            tc.tile_pool(name="const", bufs=1) as cpool,
            tc.tile_pool(name="ustage", bufs=2) as upool,
            tc.tile_pool(name="xrows", bufs=4) as xpool,
            tc.tile_pool(name="xtp", bufs=2) as xtpool,
            tc.tile_pool(name="ptr", bufs=2, space="PSUM") as ptr_pool,
            tc.tile_pool(name="pxz", bufs=2, space="PSUM") as pxz_pool,
            tc.tile_pool(name="pz", bufs=4, space="PSUM") as pz_pool,
        ):
            # ---- constants / weights ----
            U16 = cpool.tile([P, KH * G4], f8, tag="U16")
            W_sb = cpool.tile([P, 3 * G4], f16, tag="Wsb")
            b_sb = cpool.tile([P, KM], f32, tag="bsb")
            tok_sb = cpool.tile([P, T_steps * BL // P], i32, tag="tok")
            ident = cpool.tile([P, P], f32, tag="ident")
            h16 = cpool.tile([P, BL * KH], f16, tag="h16")
            cst = cpool.tile([P, BL * KH], f32, tag="cst")
            z_s = cpool.tile([P, 64], f32, tag="zs")
            a_s = cpool.tile([P, 64], f32, tag="as")
            tmp1 = cpool.tile([P, BL * KH], f32, tag="t1")
            tmp2 = cpool.tile([P, BL * KH], f32, tag="t2")
            tct = cpool.tile([P, BL * KH], f32, tag="tct")
            xz_sb = [
                cpool.tile([P, Tc * 64], f32, tag=f"xz{par}", name=f"xz{par}")
                for par in range(2)
            ]

            make_identity(nc, ident[:])

            # U (fp32 DRAM) -> U16 (fp16 SBUF), K-tile k region at cols k*G4
            for k in range(KH):
                ust = upool.tile([P, G4], f32, tag="ustage")
                nc.sync.dma_start(ust[:], U_t.ap()[k * P:(k + 1) * P, :])
                nc.vector.tensor_copy(U16[:, k * G4:(k + 1) * G4], ust[:])

            # W: 3 K-subtiles at cols kk*G4, cast to fp16 via staging
            ofs = 0
            for kk, kw in enumerate(KE_SIZES):
                wst = upool.tile([P, G4], f32, tag="ustage", name=f"wst{kk}")
                nc.sync.dma_start(wst[:kw, :], W_t.ap()[ofs:ofs + kw, :])
                nc.vector.tensor_copy(W_sb[:kw, kk * G4:(kk + 1) * G4], wst[:kw, :])
                ofs += kw

            # bias: b_sb[p, m] = b[m*128 + p]
            nc.sync.dma_start(b_sb[:], b_t.ap().rearrange("(m p) -> p m", p=P))
            nc.sync.dma_start(tok_sb[:], tok_t.ap())

            nc.gpsimd.memset(h16[:], 0.0)
            nc.gpsimd.memset(cst[:], 0.0)

            def emit_prep(c):
                """Gather + transpose + xz projection for chunk c."""
                xz_dst = xz_sb[c % 2]
                xT = xtpool.tile([P, 3 * Tc * BL], f16, tag="xT")
                for j in range(NJ):
                    xr = xpool.tile([P, E], f32, tag="xrows")
                    nc.gpsimd.indirect_dma_start(
                        out=xr[:],
                        out_offset=None,
                        in_=emb_t.ap(),
                        in_offset=IndirectOffsetOnAxis(
                            ap=tok_sb[:, c * NJ + j:c * NJ + j + 1], axis=0
                        ),
                    )
                    for kk, kw in enumerate(KE_SIZES):
                        pt = ptr_pool.tile([P, P], f32, tag="ptr")
                        nc.tensor.transpose(
                            out=pt[:kw, :], in_=xr[:, kk * P:kk * P + kw],
                            identity=ident[:],
                        )
                        nc.vector.tensor_copy(
                            xT[:kw, kk * Tc * BL + j * P:kk * Tc * BL + (j + 1) * P],
                            pt[:kw, :],
                        )
                N = Tc * BL
                for m in range(KM):
                    pxz = pxz_pool.tile([P, N], f32, tag="pxz")
                    for kk, kw in enumerate(KE_SIZES):
                        nc.tensor.matmul(
                            pxz[:],
                            W_sb[:kw, kk * G4 + m * P:kk * G4 + (m + 1) * P],
                            xT[:kw, kk * N:(kk + 1) * N],
                            start=(kk == 0),
                            stop=(kk == 2),
                        )
                    # packed dest: col = t*64 + (m%4)*16 + PERM[m//4]*4 + b
                    slot = (m % 4) * 16 + PERM[m // 4] * 4
                    dst = xz_dst[:].rearrange("p (t g) -> p t g", g=64)[
                        :, :, slot:slot + 4
                    ]
                    src = pxz[:].rearrange("p (t b) -> p t b", b=BL)
                    nc.vector.tensor_scalar_add(dst, src, b_sb[:, m:m + 1])

            # MM emission order for the last K round: group M-tiles by H-slice
            ORDER_LAST = [m for hs in range(4) for m in (hs, 4 + hs, 8 + hs, 12 + hs)]

            def emit_step_v1(c, t):
                psz = [
                    pz_pool.tile([P, 16], f32, tag="pz", name=f"pz{hs}_{c}_{t}")
                    for hs in range(4)
                ]
                for k in range(KH):
                    order = ORDER_LAST if k == KH - 1 else range(KM)
                    for m in order:
                        slot = PERM[m // 4] * 4
                        # start=True marks the whole 2KB psum bank pending-zero,
                        # so only the FIRST matmul touching each psz tile sets it
                        # (round k=0, m in 0..3); later slots overwrite via
                        # pending-zero, later k rounds accumulate.
                        nc.tensor.matmul(
                            psz[m % 4][:, slot:slot + 4],
                            U16[:, k * G4 + m * P:k * G4 + (m + 1) * P],
                            h16[:, k * BL:(k + 1) * BL],
                            start=(k == 0 and m < 4),
                            stop=(k == KH - 1),
                            skip_group_check=True,
                        )
                for hs in range(4):
                    zs = z_s[:, hs * 16:hs * 16 + 16]
                    nc.vector.tensor_add(
                        zs,
                        psz[hs][:],
                        xz_sb[c % 2][:, t * 64 + hs * 16:t * 64 + hs * 16 + 16],
                    )
                    # sigmoid over (i, f, o) slots, tanh over g slot
                    nc.scalar.activation(
                        a_s[:, hs * 16:hs * 16 + 12], z_s[:, hs * 16:hs * 16 + 12],
                        AF.Sigmoid, scale=1.0 / USC,
                    )
                    nc.scalar.activation(
                        a_s[:, hs * 16 + 12:hs * 16 + 16],
                        z_s[:, hs * 16 + 12:hs * 16 + 16],
                        AF.Tanh, scale=1.0 / USC,
                    )
                    cs = slice(hs * BL, (hs + 1) * BL)
                    nc.vector.tensor_mul(
                        tmp1[:, cs], a_s[:, hs * 16 + 4:hs * 16 + 8], cst[:, cs]
                    )  # f * c
                    nc.vector.tensor_mul(
                        tmp2[:, cs],
                        a_s[:, hs * 16:hs * 16 + 4],
                        a_s[:, hs * 16 + 12:hs * 16 + 16],
                    )  # i * g
                    nc.vector.tensor_add(cst[:, cs], tmp1[:, cs], tmp2[:, cs])
                    nc.scalar.activation(tct[:, cs], cst[:, cs], AF.Tanh)
                    nc.vector.tensor_mul(
                        h16[:, cs], a_s[:, hs * 16 + 8:hs * 16 + 12], tct[:, cs]
                    )  # h = o * tanh(c), cast to fp16 on write

            def a2(base, width):
                """2D AP over a_s/z_s: [128, (2 hs, width)] at col base within
                each 16-col hs block of the pair being processed."""
                return base.rearrange("p (hs w) -> p hs w", w=16)

            def emit_step_v2(c, t):
                # 2 psum tiles, one per hs-pair; cols = (hs%2)*16 + slot*4 + b
                psz = [
                    pz_pool.tile([P, 32], f32, tag="pz", name=f"pzp{pr}_{c}_{t}")
                    for pr in range(2)
                ]
                # pair-major PE order: all of pair 0's MMs (k-outer), then pair 1
                for pr in range(2):
                    for k in range(KH):
                        for hs in (2 * pr, 2 * pr + 1):
                            for g in range(4):
                                m = g * 4 + hs
                                slot = (hs % 2) * 16 + PERM[g] * 4
                                nc.tensor.matmul(
                                    psz[pr][:, slot:slot + 4],
                                    U16[:, k * G4 + m * P:k * G4 + (m + 1) * P],
                                    h16[:, k * BL:(k + 1) * BL],
                                    start=(k == 0 and hs == 2 * pr and g == 0),
                                    stop=(k == KH - 1),
                                    skip_group_check=True,
                                )
                xz = xz_sb[c % 2]
                for pr in range(2):
                    # per-hs adds (start as soon as that hs' slots are done)
                    for hs in (2 * pr, 2 * pr + 1):
                        nc.vector.tensor_add(
                            z_s[:, hs * 16:hs * 16 + 16],
                            psz[pr][:, (hs % 2) * 16:(hs % 2) * 16 + 16],
                            xz[:, t * 64 + hs * 16:t * 64 + hs * 16 + 16],
                        )
                    h0 = 2 * pr * 16  # base col of this pair in z_s/a_s
                    zs2 = z_s[:].rearrange("p (hs w) -> p hs w", w=16)
                    as2 = a_s[:].rearrange("p (hs w) -> p hs w", w=16)
                    # sigmoid over (i,f,o) of both hs in one 2D-AP instr
                    nc.scalar.activation(
                        as2[:, 2 * pr:2 * pr + 2, 0:12],
                        zs2[:, 2 * pr:2 * pr + 2, 0:12],
                        AF.Sigmoid, scale=1.0 / USC,
                    )
                    nc.scalar.activation(
                        as2[:, 2 * pr:2 * pr + 2, 12:16],
                        zs2[:, 2 * pr:2 * pr + 2, 12:16],
                        AF.Tanh, scale=1.0 / USC,
                    )
                    cs = slice(pr * 2 * BL, (pr + 1) * 2 * BL)  # 8 cols of cst
                    c2 = cst[:, cs].rearrange("p (hs b) -> p hs b", b=BL)
                    t1 = tmp1[:, cs].rearrange("p (hs b) -> p hs b", b=BL)
                    t2 = tmp2[:, cs].rearrange("p (hs b) -> p hs b", b=BL)
                    nc.vector.tensor_mul(
                        t1, as2[:, 2 * pr:2 * pr + 2, 4:8], c2
                    )  # f * c
                    nc.vector.tensor_mul(
                        t2,
                        as2[:, 2 * pr:2 * pr + 2, 0:4],
                        as2[:, 2 * pr:2 * pr + 2, 12:16],
                    )  # i * g
                    nc.vector.tensor_add(cst[:, cs], tmp1[:, cs], tmp2[:, cs])
                    nc.scalar.activation(tct[:, cs], cst[:, cs], AF.Tanh)
                    nc.vector.tensor_mul(
                        h16[:, cs].rearrange("p (hs b) -> p hs b", b=BL),
                        as2[:, 2 * pr:2 * pr + 2, 8:12],
                        tct[:, cs].rearrange("p (hs b) -> p hs b", b=BL),
                    )  # h = o * tanh(c), cast to fp16 on write

            def emit_step_v3(c, t):
                """Single psum bank [128, 64] for the whole step; widest ops:
                5 DVE + 3 ACT instructions per step."""
                psz = pz_pool.tile([P, 64], f32, tag="pz", name=f"pzw_{c}_{t}")
                first = True
                for k in range(KH):
                    for m in range(KM):
                        g, hs = m // 4, m % 4
                        slot = hs * 16 + PERM[g] * 4
                        nc.tensor.matmul(
                            psz[:, slot:slot + 4],
                            U16[:, k * G4 + m * P:k * G4 + (m + 1) * P],
                            h16[:, k * BL:(k + 1) * BL],
                            start=first, stop=(k == KH - 1),
                            skip_group_check=True,
                        )
                        first = False
                nc.vector.tensor_add(
                    z_s[:], psz[:], xz_sb[c % 2][:, t * 64:(t + 1) * 64]
                )
                zs3 = z_s[:].rearrange("p (hs w) -> p hs w", w=16)
                as3 = a_s[:].rearrange("p (hs w) -> p hs w", w=16)
                nc.scalar.activation(
                    as3[:, :, 0:12], zs3[:, :, 0:12], AF.Sigmoid, scale=1.0 / USC
                )
                nc.scalar.activation(
                    as3[:, :, 12:16], zs3[:, :, 12:16], AF.Tanh, scale=1.0 / USC
                )
                c3 = cst[:].rearrange("p (hs b) -> p hs b", b=BL)
                t13 = tmp1[:].rearrange("p (hs b) -> p hs b", b=BL)
                t23 = tmp2[:].rearrange("p (hs b) -> p hs b", b=BL)
                nc.vector.tensor_mul(t13, as3[:, :, 4:8], c3)
                nc.vector.tensor_mul(t23, as3[:, :, 0:4], as3[:, :, 12:16])
                nc.vector.tensor_add(cst[:], tmp1[:], tmp2[:])
                nc.scalar.activation(tct[:], cst[:], AF.Tanh)
                nc.vector.tensor_mul(
                    h16[:].rearrange("p (hs b) -> p hs b", b=BL),
                    as3[:, :, 8:12],
                    tct[:].rearrange("p (hs b) -> p hs b", b=BL),
                )

            emit_step = {"v1": emit_step_v1, "v2": emit_step_v2, "v3": emit_step_v3}[
                sched
            ]

            for rep in range(reps):
                if rep > 0:
                    nc.gpsimd.memset(h16[:], 0.0)
                    nc.gpsimd.memset(cst[:], 0.0)
                emit_prep(0)
                for c in range(nch):
                    for t in range(Tc):
                        emit_step(c, t)
                        if dbg_step is not None and (c, t) == dbg_step:
                            nc.sync.dma_start(dbg_z.ap(), z_s[:])
                            nc.sync.dma_start(dbg_h.ap(), h16[:])
                            nc.sync.dma_start(dbg_c.ap(), cst[:])
                        if t == 16 and c + 1 < nch:
                            emit_prep(c + 1)

            nc.sync.dma_start(ho_t.ap(), h16[:])
            nc.sync.dma_start(co_t.ap(), cst[:])

    return nc


_CACHE = {}

import os as _os

SCHED = _os.environ.get("KSCHED", "v2")


def _get_compiled(T_steps=T, Tc=128):
    key = (T_steps, Tc, SCHED)
    if key not in _CACHE:
        nc = bacc.Bacc(None, target_bir_lowering=False)
        build_program(nc, T_steps, Tc, sched=SCHED)
        nc.compile()
        _CACHE[key] = nc
    return _CACHE[key]


def make_tok_idx(tokens_slice, T_steps=T):
    """tokens_slice [BL, T] -> [128, T*BL/128] int32, [p, j] = t-major flat[j*128+p]."""
    flat = tokens_slice.T.reshape(-1)  # index n = t*BL + b
    return np.ascontiguousarray(
        flat.reshape(T_steps * BL // P, P).T.astype(np.int32)
    )


def unpack_state(arr):
    """[128, 16] packed (p, hs*4+b) -> [BL, H]."""
    a = np.asarray(arr).astype(np.float32).reshape(P, KH, BL)
    return a.transpose(2, 1, 0).reshape(BL, H)


def kernel(tokens, emb, W, U, b):
    tokens = np.ascontiguousarray(np.asarray(tokens), dtype=np.int32)
    emb = np.ascontiguousarray(np.asarray(emb), dtype=np.float32)
    W = np.ascontiguousarray(np.asarray(W), dtype=np.float32)
    U = np.ascontiguousarray(np.asarray(U), dtype=np.float32)
    b = np.ascontiguousarray(np.asarray(b), dtype=np.float32)

    nc = _get_compiled()
    Ws, Us, bs = W * USC, U * USC, b * USC
    in_maps = []
    for i in range(NCORES):
        in_maps.append(
            {
                "emb": emb,
                "W": Ws,
                "U": Us,
                "bvec": bs,
                "tok": make_tok_idx(tokens[i * BL:(i + 1) * BL]),
            }
        )
    res = run_bass_kernel_spmd(nc, in_maps, core_ids=list(range(NCORES))).results

    h = np.zeros((B, H), np.float32)
    c = np.zeros((B, H), np.float32)
    for i in range(NCORES):
        h[i * BL:(i + 1) * BL] = unpack_state(res[i]["ho"])
        c[i * BL:(i + 1) * BL] = unpack_state(res[i]["co"])
    return h, c


def _build_run_fn(nc):
    """jit'd fn running the kernel once on 8 cores (device-resident args)."""
    import jax
    from jax.sharding import Mesh, PartitionSpec
    from jax.experimental.shard_map import shard_map
    import concourse.mybir as mybir_
    from concourse import bass2jax

    bass2jax.install_neuronx_cc_hook()

    partition_name = nc.partition_id_tensor.name if nc.partition_id_tensor else None
    in_names, out_names, out_avals = [], [], []
    for alloc in nc.m.functions[0].allocations:
        if not isinstance(alloc, mybir_.MemoryLocationSet):
            continue
        name = alloc.memorylocations[0].name
        if alloc.kind == "ExternalInput":
            if name != partition_name:
                in_names.append(name)
        elif alloc.kind == "ExternalOutput":
            out_names.append(name)
            out_avals.append(
                jax.core.ShapedArray(
                    tuple(alloc.tensor_shape), mybir_.dt.np(alloc.dtype)
                )
            )
    n_params = len(in_names)
    all_in_names = list(in_names) + list(out_names)
    if partition_name is not None:
        all_in_names.append(partition_name)

    def _body(*args):
        operands = list(args)
        if partition_name is not None:
            operands.append(bass2jax.partition_id_tensor())
        return tuple(
            bass2jax._bass_exec_p.bind(
                *operands,
                out_avals=tuple(out_avals),
                in_names=tuple(all_in_names),
                out_names=tuple(out_names),
                lowering_input_output_aliases=(),
                sim_require_finite=True,
                sim_require_nnan=True,
                nc=nc,
            )
        )

    devices = jax.devices()[:NCORES]
    mesh = Mesh(np.asarray(devices), ("core",))
    nio = n_params + len(out_names)
    fn = jax.jit(
        shard_map(
            _body,
            mesh=mesh,
            in_specs=(PartitionSpec("core"),) * nio,
            out_specs=(PartitionSpec("core"),) * len(out_names),
            check_rep=False,
        )
    )
    return fn, in_names, out_names, out_avals


def _prep_fn(nc, in_maps):
    """Build jit fn + device-resident args for nc; warm it once."""
    import jax

    fn, in_names, out_names, out_avals = _build_run_fn(nc)
    concat_in = [
        np.concatenate([in_maps[c][k] for c in range(NCORES)], axis=0)
        for k in in_names
    ]
    concat_zeros = [
        np.zeros((NCORES * a.shape[0], *a.shape[1:]), a.dtype) for a in out_avals
    ]
    args = [jax.device_put(x) for x in concat_in + concat_zeros]
    jax.block_until_ready(fn(*args))  # compile + first exec
    return fn, args


def _make_in_maps(np_inputs):
    tokens = np.ascontiguousarray(np.asarray(np_inputs["tokens"]), dtype=np.int32)
    in_maps = []
    for i in range(NCORES):
        in_maps.append(
            {
                "emb": np.asarray(np_inputs["emb"], np.float32),
                "W": np.asarray(np_inputs["W"], np.float32) * USC,
                "U": np.asarray(np_inputs["U"], np.float32) * USC,
                "bvec": np.asarray(np_inputs["b"], np.float32) * USC,
                "tok": make_tok_idx(tokens[i * BL:(i + 1) * BL]),
            }
        )
    return in_maps


def time_kernel_hw(np_inputs, reps_hi=3, calls=10, burst=16):
    """Estimate one-pass HW time (ns), robust to tunnel noise/drift.

    Two estimators, both from interleaved trials so slow drift cancels:
      A) amplified-variant delta: same program with the compute repeated
         1x vs reps_hi x, min-wall delta / (reps_hi - 1).
      B) async-burst delta: launch `burst+1` vs 1 executions of the R1
         program back-to-back (block once at the end) — the burst
         pipelines on device, so delta/burst ~ one pass.
    Returns the smaller positive estimate (both are upward-biased by any
    non-cancelled overhead).
    """
    import time as _time
    import jax

    in_maps = _make_in_maps(np_inputs)

    nc1 = _get_compiled()
    nc_hi = bacc.Bacc(None, target_bir_lowering=False)
    build_program(nc_hi, T, 128, reps=reps_hi, sched=SCHED)
    nc_hi.compile()

    fn1, args1 = _prep_fn(nc1, in_maps)
    fnh, argsh = _prep_fn(nc_hi, in_maps)

    w1, wh, wb = [], [], []
    for _ in range(calls):
        t0 = _time.perf_counter()
        jax.block_until_ready(fn1(*args1))
        w1.append(_time.perf_counter() - t0)

        t0 = _time.perf_counter()
        jax.block_until_ready(fnh(*argsh))
        wh.append(_time.perf_counter() - t0)

        t0 = _time.perf_counter()
        outs = [fn1(*args1) for _ in range(burst + 1)]
        jax.block_until_ready(outs[-1])
        wb.append(_time.perf_counter() - t0)

    m1, mh, mb = min(w1), min(wh), min(wb)
    est_a = (mh - m1) / (reps_hi - 1)
    est_b = (mb - m1) / burst
    print(
        f"timing: R1 min {m1 * 1e3:.3f} ms, R{reps_hi} min {mh * 1e3:.3f} ms, "
        f"burst{burst + 1} min {mb * 1e3:.3f} ms -> "
        f"estA {est_a * 1e6:.1f} us, estB {est_b * 1e6:.1f} us"
    )
    cands = [e for e in (est_a, est_b) if e > 0]
    return (min(cands) if cands else 0.0) * 1e9




# revision 23
# speedup vs baseline: 1.4202x; 1.4202x over previous
"""
Trainium2 Bass kernel for nn_Encoder (embedding lookup + LSTM, returns final (h, c)).

Strategy (data-parallel over batch, per sharding hint):
  - 8 cores, each handles B_local = 4 of the 32 batch rows.
  - Per core: gather embedding rows via indirect DMA (t-major order),
    transpose on PE, project x @ W with fp32r matmuls (chunked over T),
    then run the 512-step recurrence with U as the stationary operand
    in fp16 (FWL 2x weight loads) producing gates transposed
    (4H on partitions) so activations/cell update run wide on ACT/DVE.
  - Gate layout: psum z tile per H-slice hs (4 of them, one PSUM bank each),
    packed columns (gate', b) with gate' order (i, f, o, g) so one sigmoid
    covers i,f,o and one tanh covers g.
  - h is kept as hT [128 x (hs, b)] fp16 which is exactly the moving-operand
    layout the next step's matmuls need.

Host side: shard/marshal inputs, run SPMD on 8 cores, unpack outputs.
"""

import numpy as np

import concourse.bass as bass
import concourse.mybir as mybir
import concourse.tile as tile
from concourse import bacc
from concourse.bass import IndirectOffsetOnAxis
from concourse.bass_utils import run_bass_kernel_spmd
from concourse.masks import make_identity

# Problem constants (hardcoded; harness contract)
B, T, V, E, H = 32, 512, 20000, 300, 512
G4 = 4 * H            # 2048
NCORES = 8
BL = B // NCORES      # 4 batch rows per core
P = 128
KM = G4 // P          # 16 M-tiles over 4H
KH = H // P           # 4 K-tiles over H
KE_SIZES = [128, 128, 44]   # K subtiles over E=300
# Keras gate g (i,f,g,o) -> packed slot (i,f,o,g): sigmoid = slots 0..2, tanh = slot 3
PERM = [0, 1, 3, 2]

f32 = mybir.dt.float32
f32r = mybir.dt.float32r
f16 = mybir.dt.float16
f8 = mybir.dt.float8e3
i32 = mybir.dt.int32

# U (and W, b) are pre-scaled by USC on the host so that U fits the fp8e3
# (e3m4) normal range: raw |U| ~ N(0, 1/sqrt(H)) lies below the 0.25 min
# normal, 32x shifts it into [~0.25, 15.5]. z comes out of PSUM scaled by
# USC; the gate activations undo it via their input scale (exact power of 2).
USC = 32.0

AF = mybir.ActivationFunctionType


def build_program(nc, T_steps=T, Tc=128, dbg_step=None, reps=1, sched="v2",
                  hw_loop=False):
    """Emit the full per-core program into nc (a bacc.Bacc).

    reps > 1 repeats the whole compute (for timing amplification);
    hw_loop=True wraps the repetition in a hardware For_i loop (body
    emitted once, so compile time is independent of reps)."""
    assert T_steps % Tc == 0
    nch = T_steps // Tc
    NJ = Tc * BL // P  # gathers (128-row groups) per chunk

    emb_t = nc.declare_dram_parameter("emb", [V, E], f32, isOutput=False)
    W_t = nc.declare_dram_parameter("W", [E, G4], f32, isOutput=False)
    U_t = nc.declare_dram_parameter("U", [H, G4], f32, isOutput=False)
    b_t = nc.declare_dram_parameter("bvec", [G4], f32, isOutput=False)
    tok_t = nc.declare_dram_parameter("tok", [P, T_steps * BL // P], i32, isOutput=False)
    ho_t = nc.declare_dram_parameter("ho", [P, BL * KH], f16, isOutput=True)
    co_t = nc.declare_dram_parameter("co", [P, BL * KH], f32, isOutput=True)
    if dbg_step is not None:
        dbg_z = nc.declare_dram_parameter("dbg_z", [P, 64], f32, isOutput=True)
        dbg_h = nc.declare_dram_parameter("dbg_h", [P, BL * KH], f16, isOutput=True)
        dbg_c = nc.declare_dram_parameter("dbg_c", [P, BL * KH], f32, isOutput=True)

    with tile.TileContext(nc) as tc:
        with (
            tc.tile_pool(name="const", bufs=1) as cpool,
            tc.tile_pool(name="ustage", bufs=2) as upool,
            tc.tile_pool(name="xrows", bufs=4) as xpool,
            tc.tile_pool(name="xtp", bufs=2) as xtpool,
            tc.tile_pool(name="ptr", bufs=2, space="PSUM") as ptr_pool,
            tc.tile_pool(name="pxz", bufs=2, space="PSUM") as pxz_pool,
            tc.tile_pool(name="pz", bufs=4, space="PSUM") as pz_pool,
        ):
            # ---- constants / weights ----
            U16 = cpool.tile([P, KH * G4], f8, tag="U16")
            W_sb = cpool.tile([P, 3 * G4], f16, tag="Wsb")
            b_sb = cpool.tile([P, KM], f32, tag="bsb")
            tok_sb = cpool.tile([P, T_steps * BL // P], i32, tag="tok")
            ident = cpool.tile([P, P], f32, tag="ident")
            h16 = cpool.tile([P, BL * KH], f16, tag="h16")
            cst = cpool.tile([P, BL * KH], f32, tag="cst")
            z_s = cpool.tile([P, 64], f32, tag="zs")
            a_s = cpool.tile([P, 64], f32, tag="as")
            tmp1 = cpool.tile([P, BL * KH], f32, tag="t1")
            tmp2 = cpool.tile([P, BL * KH], f32, tag="t2")
            tct = cpool.tile([P, BL * KH], f32, tag="tct")
            # fp16: halves SBUF and doubles as the moving operand of the
            # v4 identity-fold matmul (fp32 moving can't pair with fp8 lhsT)
            xz_sb = [
                cpool.tile([P, Tc * 64], f16, tag=f"xz{par}", name=f"xz{par}")
                for par in range(2)
            ]

            make_identity(nc, ident[:])
            ident8 = cpool.tile([P, P], f8, tag="ident8")
            g_s = cpool.tile([P, 16], f32, tag="gs")
            nc.vector.tensor_copy(ident8[:], ident[:])

            # U (fp32 DRAM) -> U16 (fp16 SBUF), K-tile k region at cols k*G4
            for k in range(KH):
                ust = upool.tile([P, G4], f32, tag="ustage")
                nc.sync.dma_start(ust[:], U_t.ap()[k * P:(k + 1) * P, :])
                nc.vector.tensor_copy(U16[:, k * G4:(k + 1) * G4], ust[:])

            # W: 3 K-subtiles at cols kk*G4, cast to fp16 via staging
            ofs = 0
            for kk, kw in enumerate(KE_SIZES):
                wst = upool.tile([P, G4], f32, tag="ustage", name=f"wst{kk}")
                nc.sync.dma_start(wst[:kw, :], W_t.ap()[ofs:ofs + kw, :])
                nc.vector.tensor_copy(W_sb[:kw, kk * G4:(kk + 1) * G4], wst[:kw, :])
                ofs += kw

            # bias: b_sb[p, m] = b[m*128 + p]
            nc.sync.dma_start(b_sb[:], b_t.ap().rearrange("(m p) -> p m", p=P))
            nc.sync.dma_start(tok_sb[:], tok_t.ap())

            def emit_prep(c):
                """Gather + transpose + xz projection for chunk c."""
                xz_dst = xz_sb[c % 2]
                xT = xtpool.tile([P, 3 * Tc * BL], f16, tag="xT")
                for j in range(NJ):
                    xr = xpool.tile([P, E], f32, tag="xrows")
                    nc.gpsimd.indirect_dma_start(
                        out=xr[:],
                        out_offset=None,
                        in_=emb_t.ap(),
                        in_offset=IndirectOffsetOnAxis(
                            ap=tok_sb[:, c * NJ + j:c * NJ + j + 1], axis=0
                        ),
                    )
                    for kk, kw in enumerate(KE_SIZES):
                        pt = ptr_pool.tile([P, P], f32, tag="ptr")
                        nc.tensor.transpose(
                            out=pt[:kw, :], in_=xr[:, kk * P:kk * P + kw],
                            identity=ident[:],
                        )
                        nc.vector.tensor_copy(
                            xT[:kw, kk * Tc * BL + j * P:kk * Tc * BL + (j + 1) * P],
                            pt[:kw, :],
                        )
                N = Tc * BL
                for m in range(KM):
                    pxz = pxz_pool.tile([P, N], f32, tag="pxz")
                    for kk, kw in enumerate(KE_SIZES):
                        nc.tensor.matmul(
                            pxz[:],
                            W_sb[:kw, kk * G4 + m * P:kk * G4 + (m + 1) * P],
                            xT[:kw, kk * N:(kk + 1) * N],
                            start=(kk == 0),
                            stop=(kk == 2),
                        )
                    # packed dest: col = t*64 + (m%4)*16 + PERM[m//4]*4 + b
                    slot = (m % 4) * 16 + PERM[m // 4] * 4
                    dst = xz_dst[:].rearrange("p (t g) -> p t g", g=64)[
                        :, :, slot:slot + 4
                    ]
                    src = pxz[:].rearrange("p (t b) -> p t b", b=BL)
                    nc.vector.tensor_scalar_add(dst, src, b_sb[:, m:m + 1])

            # MM emission order for the last K round: group M-tiles by H-slice
            ORDER_LAST = [m for hs in range(4) for m in (hs, 4 + hs, 8 + hs, 12 + hs)]

            def emit_step_v1(c, t):
                psz = [
                    pz_pool.tile([P, 16], f32, tag="pz", name=f"pz{hs}_{c}_{t}")
                    for hs in range(4)
                ]
                for k in range(KH):
                    order = ORDER_LAST if k == KH - 1 else range(KM)
                    for m in order:
                        slot = PERM[m // 4] * 4
                        # start=True marks the whole 2KB psum bank pending-zero,
                        # so only the FIRST matmul touching each psz tile sets it
                        # (round k=0, m in 0..3); later slots overwrite via
                        # pending-zero, later k rounds accumulate.
                        nc.tensor.matmul(
                            psz[m % 4][:, slot:slot + 4],
                            U16[:, k * G4 + m * P:k * G4 + (m + 1) * P],
                            h16[:, k * BL:(k + 1) * BL],
                            start=(k == 0 and m < 4),
                            stop=(k == KH - 1),
                            skip_group_check=True,
                        )
                for hs in range(4):
                    zs = z_s[:, hs * 16:hs * 16 + 16]
                    nc.vector.tensor_add(
                        zs,
                        psz[hs][:],
                        xz_sb[c % 2][:, t * 64 + hs * 16:t * 64 + hs * 16 + 16],
                    )
                    # sigmoid over (i, f, o) slots, tanh over g slot
                    nc.scalar.activation(
                        a_s[:, hs * 16:hs * 16 + 12], z_s[:, hs * 16:hs * 16 + 12],
                        AF.Sigmoid, scale=1.0 / USC,
                    )
                    nc.scalar.activation(
                        a_s[:, hs * 16 + 12:hs * 16 + 16],
                        z_s[:, hs * 16 + 12:hs * 16 + 16],
                        AF.Tanh, scale=1.0 / USC,
                    )
                    cs = slice(hs * BL, (hs + 1) * BL)
                    nc.vector.tensor_mul(
                        tmp1[:, cs], a_s[:, hs * 16 + 4:hs * 16 + 8], cst[:, cs]
                    )  # f * c
                    nc.vector.tensor_mul(
                        tmp2[:, cs],
                        a_s[:, hs * 16:hs * 16 + 4],
                        a_s[:, hs * 16 + 12:hs * 16 + 16],
                    )  # i * g
                    nc.vector.tensor_add(cst[:, cs], tmp1[:, cs], tmp2[:, cs])
                    nc.scalar.activation(tct[:, cs], cst[:, cs], AF.Tanh)
                    nc.vector.tensor_mul(
                        h16[:, cs], a_s[:, hs * 16 + 8:hs * 16 + 12], tct[:, cs]
                    )  # h = o * tanh(c), cast to fp16 on write

            def a2(base, width):
                """2D AP over a_s/z_s: [128, (2 hs, width)] at col base within
                each 16-col hs block of the pair being processed."""
                return base.rearrange("p (hs w) -> p hs w", w=16)

            def emit_step_v2(c, t):
                # 2 psum tiles, one per hs-pair; cols = (hs%2)*16 + slot*4 + b
                psz = [
                    pz_pool.tile([P, 32], f32, tag="pz", name=f"pzp{pr}_{c}_{t}")
                    for pr in range(2)
                ]
                # pair-major PE order: all of pair 0's MMs (k-outer), then pair 1
                for pr in range(2):
                    for k in range(KH):
                        for hs in (2 * pr, 2 * pr + 1):
                            for g in range(4):
                                m = g * 4 + hs
                                slot = (hs % 2) * 16 + PERM[g] * 4
                                nc.tensor.matmul(
                                    psz[pr][:, slot:slot + 4],
                                    U16[:, k * G4 + m * P:k * G4 + (m + 1) * P],
                                    h16[:, k * BL:(k + 1) * BL],
                                    start=(k == 0 and hs == 2 * pr and g == 0),
                                    stop=(k == KH - 1),
                                    skip_group_check=True,
                                )
                xz = xz_sb[c % 2]
                for pr in range(2):
                    # per-hs adds (start as soon as that hs' slots are done)
                    for hs in (2 * pr, 2 * pr + 1):
                        nc.vector.tensor_add(
                            z_s[:, hs * 16:hs * 16 + 16],
                            psz[pr][:, (hs % 2) * 16:(hs % 2) * 16 + 16],
                            xz[:, t * 64 + hs * 16:t * 64 + hs * 16 + 16],
                        )
                    h0 = 2 * pr * 16  # base col of this pair in z_s/a_s
                    zs2 = z_s[:].rearrange("p (hs w) -> p hs w", w=16)
                    as2 = a_s[:].rearrange("p (hs w) -> p hs w", w=16)
                    # sigmoid over (i,f,o) of both hs in one 2D-AP instr
                    nc.scalar.activation(
                        as2[:, 2 * pr:2 * pr + 2, 0:12],
                        zs2[:, 2 * pr:2 * pr + 2, 0:12],
                        AF.Sigmoid, scale=1.0 / USC,
                    )
                    nc.scalar.activation(
                        as2[:, 2 * pr:2 * pr + 2, 12:16],
                        zs2[:, 2 * pr:2 * pr + 2, 12:16],
                        AF.Tanh, scale=1.0 / USC,
                    )
                    cs = slice(pr * 2 * BL, (pr + 1) * 2 * BL)  # 8 cols of cst
                    c2 = cst[:, cs].rearrange("p (hs b) -> p hs b", b=BL)
                    t1 = tmp1[:, cs].rearrange("p (hs b) -> p hs b", b=BL)
                    t2 = tmp2[:, cs].rearrange("p (hs b) -> p hs b", b=BL)
                    nc.vector.tensor_mul(
                        t1, as2[:, 2 * pr:2 * pr + 2, 4:8], c2
                    )  # f * c
                    nc.vector.tensor_mul(
                        t2,
                        as2[:, 2 * pr:2 * pr + 2, 0:4],
                        as2[:, 2 * pr:2 * pr + 2, 12:16],
                    )  # i * g
                    nc.vector.tensor_add(cst[:, cs], tmp1[:, cs], tmp2[:, cs])
                    nc.scalar.activation(tct[:, cs], cst[:, cs], AF.Tanh)
                    nc.vector.tensor_mul(
                        h16[:, cs].rearrange("p (hs b) -> p hs b", b=BL),
                        as2[:, 2 * pr:2 * pr + 2, 8:12],
                        tct[:, cs].rearrange("p (hs b) -> p hs b", b=BL),
                    )  # h = o * tanh(c), cast to fp16 on write

            def emit_step_v3(c, t):
                """Single psum bank [128, 64] for the whole step; widest ops:
                5 DVE + 3 ACT instructions per step."""
                psz = pz_pool.tile([P, 64], f32, tag="pz", name=f"pzw_{c}_{t}")
                first = True
                for k in range(KH):
                    for m in range(KM):
                        g, hs = m // 4, m % 4
                        slot = hs * 16 + PERM[g] * 4
                        nc.tensor.matmul(
                            psz[:, slot:slot + 4],
                            U16[:, k * G4 + m * P:k * G4 + (m + 1) * P],
                            h16[:, k * BL:(k + 1) * BL],
                            start=first, stop=(k == KH - 1),
                            skip_group_check=True,
                        )
                        first = False
                nc.vector.tensor_add(
                    z_s[:], psz[:], xz_sb[c % 2][:, t * 64:(t + 1) * 64]
                )
                zs3 = z_s[:].rearrange("p (hs w) -> p hs w", w=16)
                as3 = a_s[:].rearrange("p (hs w) -> p hs w", w=16)
                nc.scalar.activation(
                    as3[:, :, 0:12], zs3[:, :, 0:12], AF.Sigmoid, scale=1.0 / USC
                )
                nc.scalar.activation(
                    as3[:, :, 12:16], zs3[:, :, 12:16], AF.Tanh, scale=1.0 / USC
                )
                c3 = cst[:].rearrange("p (hs b) -> p hs b", b=BL)
                t13 = tmp1[:].rearrange("p (hs b) -> p hs b", b=BL)
                t23 = tmp2[:].rearrange("p (hs b) -> p hs b", b=BL)
                nc.vector.tensor_mul(t13, as3[:, :, 4:8], c3)
                nc.vector.tensor_mul(t23, as3[:, :, 0:4], as3[:, :, 12:16])
                nc.vector.tensor_add(cst[:], tmp1[:], tmp2[:])
                nc.scalar.activation(tct[:], cst[:], AF.Tanh)
                nc.vector.tensor_mul(
                    h16[:].rearrange("p (hs b) -> p hs b", b=BL),
                    as3[:, :, 8:12],
                    tct[:].rearrange("p (hs b) -> p hs b", b=BL),
                )

            def emit_step_v4(c, t):
                """Minimal loop-carried latency:
                  - xz folded into PSUM via one identity matmul (no DVE z-add;
                    sigmoid reads PSUM directly — faster ScE port),
                  - tanh(g) = 2*sigmoid(2x)-1: host pre-scales g-columns of
                    U/W/b by 2, so ONE sigmoid covers all 4 gates, then a
                    fused (mult,add) tensor_scalar recovers g.
                Per step: 65 MMs, 5 DVE, 2 ACT."""
                psz = pz_pool.tile([P, 64], f32, tag="pz", name=f"pzw_{c}_{t}")
                first = True
                for k in range(KH):
                    for m in range(KM):
                        g, hs = m // 4, m % 4
                        slot = hs * 16 + PERM[g] * 4
                        nc.tensor.matmul(
                            psz[:, slot:slot + 4],
                            U16[:, k * G4 + m * P:k * G4 + (m + 1) * P],
                            h16[:, k * BL:(k + 1) * BL],
                            start=first, stop=False,
                            skip_group_check=True,
                        )
                        first = False
                # psz += I @ xz_t  (all 64 cols in one matmul, N=64 moving)
                nc.tensor.matmul(
                    psz[:],
                    ident8[:],
                    xz_sb[c % 2][:, t * 64:(t + 1) * 64],
                    start=False, stop=True,
                    skip_group_check=True,
                )
                as3 = a_s[:].rearrange("p (hs w) -> p hs w", w=16)
                nc.scalar.activation(a_s[:], psz[:], AF.Sigmoid, scale=1.0 / USC)
                g3 = g_s[:].rearrange("p (hs b) -> p hs b", b=BL)
                nc.vector.tensor_scalar(
                    g3, as3[:, :, 12:16], 2.0, -1.0,
                    mybir.AluOpType.mult, mybir.AluOpType.add,
                )
                c3 = cst[:].rearrange("p (hs b) -> p hs b", b=BL)
                t13 = tmp1[:].rearrange("p (hs b) -> p hs b", b=BL)
                t23 = tmp2[:].rearrange("p (hs b) -> p hs b", b=BL)
                nc.vector.tensor_mul(t13, as3[:, :, 4:8], c3)
                nc.vector.tensor_mul(t23[:, :, :], as3[:, :, 0:4], g3)
                nc.vector.tensor_add(cst[:], tmp1[:], tmp2[:])
                nc.scalar.activation(tct[:], cst[:], AF.Tanh)
                nc.vector.tensor_mul(
                    h16[:].rearrange("p (hs b) -> p hs b", b=BL),
                    as3[:, :, 8:12],
                    tct[:].rearrange("p (hs b) -> p hs b", b=BL),
                )

            emit_step = {
                "v1": emit_step_v1, "v2": emit_step_v2,
                "v3": emit_step_v3, "v4": emit_step_v4,
            }[sched]

            def emit_body():
                nc.gpsimd.memset(h16[:], 0.0)
                nc.gpsimd.memset(cst[:], 0.0)
                emit_prep(0)
                for c in range(nch):
                    for t in range(Tc):
                        emit_step(c, t)
                        if dbg_step is not None and (c, t) == dbg_step:
                            nc.sync.dma_start(dbg_z.ap(), z_s[:])
                            nc.sync.dma_start(dbg_h.ap(), h16[:])
                            nc.sync.dma_start(dbg_c.ap(), cst[:])
                        if t == 16 and c + 1 < nch:
                            emit_prep(c + 1)

            if hw_loop:
                with tc.For_i(0, reps):
                    emit_body()
            else:
                for rep in range(reps):
                    emit_body()

            nc.sync.dma_start(ho_t.ap(), h16[:])
            nc.sync.dma_start(co_t.ap(), cst[:])

    return nc


_CACHE = {}

import os as _os

SCHED = _os.environ.get("KSCHED", "v2")


def _get_compiled(T_steps=T, Tc=128):
    key = (T_steps, Tc, SCHED)
    if key not in _CACHE:
        nc = bacc.Bacc(None, target_bir_lowering=False)
        build_program(nc, T_steps, Tc, sched=SCHED)
        nc.compile()
        _CACHE[key] = nc
    return _CACHE[key]


def _scale_params(W, U, b):
    """Pre-scale W/U/b by USC (and g-gate columns by an extra 2 for the
    v4 tanh-as-sigmoid trick); clip U to the fp8e3 normal range."""
    Ws = np.asarray(W, np.float32) * USC
    Us = np.asarray(U, np.float32) * USC
    bs = np.asarray(b, np.float32) * USC
    if SCHED == "v4":
        Ws[:, 2 * H:3 * H] *= 2.0
        Us[:, 2 * H:3 * H] *= 2.0
        bs[2 * H:3 * H] *= 2.0
    np.clip(Us, -15.5, 15.5, out=Us)
    return Ws, Us, bs


def make_tok_idx(tokens_slice, T_steps=T):
    """tokens_slice [BL, T] -> [128, T*BL/128] int32, [p, j] = t-major flat[j*128+p]."""
    flat = tokens_slice.T.reshape(-1)  # index n = t*BL + b
    return np.ascontiguousarray(
        flat.reshape(T_steps * BL // P, P).T.astype(np.int32)
    )


def unpack_state(arr):
    """[128, 16] packed (p, hs*4+b) -> [BL, H]."""
    a = np.asarray(arr).astype(np.float32).reshape(P, KH, BL)
    return a.transpose(2, 1, 0).reshape(BL, H)


def kernel(tokens, emb, W, U, b):
    tokens = np.ascontiguousarray(np.asarray(tokens), dtype=np.int32)
    emb = np.ascontiguousarray(np.asarray(emb), dtype=np.float32)
    W = np.ascontiguousarray(np.asarray(W), dtype=np.float32)
    U = np.ascontiguousarray(np.asarray(U), dtype=np.float32)
    b = np.ascontiguousarray(np.asarray(b), dtype=np.float32)

    nc = _get_compiled()
    Ws, Us, bs = _scale_params(W, U, b)
    in_maps = []
    for i in range(NCORES):
        in_maps.append(
            {
                "emb": emb,
                "W": Ws,
                "U": Us,
                "bvec": bs,
                "tok": make_tok_idx(tokens[i * BL:(i + 1) * BL]),
            }
        )
    res = run_bass_kernel_spmd(nc, in_maps, core_ids=list(range(NCORES))).results

    h = np.zeros((B, H), np.float32)
    c = np.zeros((B, H), np.float32)
    for i in range(NCORES):
        h[i * BL:(i + 1) * BL] = unpack_state(res[i]["ho"])
        c[i * BL:(i + 1) * BL] = unpack_state(res[i]["co"])
    return h, c


def _build_run_fn(nc):
    """jit'd fn running the kernel once on 8 cores (device-resident args)."""
    import jax
    from jax.sharding import Mesh, PartitionSpec
    from jax.experimental.shard_map import shard_map
    import concourse.mybir as mybir_
    from concourse import bass2jax

    bass2jax.install_neuronx_cc_hook()

    partition_name = nc.partition_id_tensor.name if nc.partition_id_tensor else None
    in_names, out_names, out_avals = [], [], []
    for alloc in nc.m.functions[0].allocations:
        if not isinstance(alloc, mybir_.MemoryLocationSet):
            continue
        name = alloc.memorylocations[0].name
        if alloc.kind == "ExternalInput":
            if name != partition_name:
                in_names.append(name)
        elif alloc.kind == "ExternalOutput":
            out_names.append(name)
            out_avals.append(
                jax.core.ShapedArray(
                    tuple(alloc.tensor_shape), mybir_.dt.np(alloc.dtype)
                )
            )
    n_params = len(in_names)
    all_in_names = list(in_names) + list(out_names)
    if partition_name is not None:
        all_in_names.append(partition_name)

    def _body(*args):
        operands = list(args)
        if partition_name is not None:
            operands.append(bass2jax.partition_id_tensor())
        return tuple(
            bass2jax._bass_exec_p.bind(
                *operands,
                out_avals=tuple(out_avals),
                in_names=tuple(all_in_names),
                out_names=tuple(out_names),
                lowering_input_output_aliases=(),
                sim_require_finite=True,
                sim_require_nnan=True,
                nc=nc,
            )
        )

    devices = jax.devices()[:NCORES]
    mesh = Mesh(np.asarray(devices), ("core",))
    nio = n_params + len(out_names)
    fn = jax.jit(
        shard_map(
            _body,
            mesh=mesh,
            in_specs=(PartitionSpec("core"),) * nio,
            out_specs=(PartitionSpec("core"),) * len(out_names),
            check_rep=False,
        )
    )
    return fn, in_names, out_names, out_avals


def _prep_fn(nc, in_maps):
    """Build jit fn + device-resident args for nc; warm it once."""
    import jax

    fn, in_names, out_names, out_avals = _build_run_fn(nc)
    concat_in = [
        np.concatenate([in_maps[c][k] for c in range(NCORES)], axis=0)
        for k in in_names
    ]
    concat_zeros = [
        np.zeros((NCORES * a.shape[0], *a.shape[1:]), a.dtype) for a in out_avals
    ]
    args = [jax.device_put(x) for x in concat_in + concat_zeros]
    jax.block_until_ready(fn(*args))  # compile + first exec
    return fn, args


def _make_in_maps(np_inputs):
    tokens = np.ascontiguousarray(np.asarray(np_inputs["tokens"]), dtype=np.int32)
    Ws, Us, bs = _scale_params(
        np_inputs["W"], np_inputs["U"], np_inputs["b"]
    )
    in_maps = []
    for i in range(NCORES):
        in_maps.append(
            {
                "emb": np.asarray(np_inputs["emb"], np.float32),
                "W": Ws,
                "U": Us,
                "bvec": bs,
                "tok": make_tok_idx(tokens[i * BL:(i + 1) * BL]),
            }
        )
    return in_maps


def time_kernel_hw(np_inputs, r_lo=1, r_hi=129, calls=8):
    """Estimate one-pass HW time (ns) via hardware-loop amplification.

    Two programs, identical bodies inside a For_i hardware loop with trip
    counts r_lo and r_hi. The wall delta (interleaved mins, so tunnel
    drift cancels) divided by (r_hi - r_lo) bodies gives one pass. With
    128 extra bodies the signal is ~100ms, far above tunnel jitter.
    """
    import time as _time
    import jax

    in_maps = _make_in_maps(np_inputs)

    ncs = {}
    for r in (r_lo, r_hi):
        nc = bacc.Bacc(None, target_bir_lowering=False)
        build_program(nc, T, 128, reps=r, sched=SCHED, hw_loop=True)
        nc.compile()
        ncs[r] = nc

    fn_lo, args_lo = _prep_fn(ncs[r_lo], in_maps)
    fn_hi, args_hi = _prep_fn(ncs[r_hi], in_maps)

    wl, wh = [], []
    for _ in range(calls):
        t0 = _time.perf_counter()
        jax.block_until_ready(fn_lo(*args_lo))
        wl.append(_time.perf_counter() - t0)

        t0 = _time.perf_counter()
        jax.block_until_ready(fn_hi(*args_hi))
        wh.append(_time.perf_counter() - t0)

    ml, mh = min(wl), min(wh)
    est = (mh - ml) / (r_hi - r_lo)
    print(
        f"timing: R{r_lo} min {ml * 1e3:.3f} ms, R{r_hi} min {mh * 1e3:.3f} ms "
        f"-> per-pass {est * 1e6:.2f} us"
    )
    return max(est, 0.0) * 1e9


